# revision 3
# baseline (speedup 1.0000x reference)
"""Trainium2 Bass kernel for nn_MultiHeadAttention_34162169872901.

MultiHeadAttention (B=4, S=2048, d_model=512, 8 heads, d_k=64) with a
relative-position bias table (511 entries, clamp +-255) and an all-ones mask.

Sharding (8 NeuronCores): core c handles batch b = c//2 and 4 of the 8 heads
(c%2 selects the head half); the host sums the two partial outputs per batch.

Design (v2, ~199us vs 264us v1 baseline):
  - Pacing engine is the Scalar/ACT exp stream (128 activations of
    [128, 1024] over PSUM, ~1.2us each incl. semaphore overhead).  The
    scores->exp->(bias mul)->AV chain is software-pipelined per
    (k-tile-pair, head) item with the AV matmuls lagging LOOK=3 items so the
    PE and ACT engines overlap instead of alternating.
  - Every matmul runs in the full 128x128 array mode: per-head K=64 score
    contractions, the 1/l broadcast, and the d_k=64 O-projection are all
    zero-padded to K=128 (zeroed Q tiles / cx slots / all-ones stationary),
    because switching between row/col-tiled and full PE modes costs ~90ns
    per transition (~20us total when scores/AV/O-proj alternate).
  - Non-critical work (V projection, Q projections for later q-chunks, the
    previous block's normalization and O-projection) is emitted as pending
    units between pipeline items, keeping the PE FIFO full without blocking
    it on late DMAs (Q-proj units flush late; their xq chunks arrive late).
  - exp(s - 3) keeps f16 softmax sums in range (softmax is shift-invariant;
    the denominator comes from a 65th all-ones V column, its reciprocal is
    broadcast across partitions by an all-ones matmul).
  - Input DMAs ride 2 rings (sync HWDGE + gpsimd SWDGE) in first-use order;
    the scalar/ACT ring stays DMA-free (each DMA instr costs ~1.7us of ACT
    queue time).  Outputs alternate rings; the last block's tail is inlined
    with in-band groups first and ACT-assisted output casts.
  - All-f16 data path (f32 PSUM accumulation).  fp8e4 DoubleRow AV was tried
    and works but costs ~3% output error (over the 2e-2 gate); the fp8
    branch was removed.
"""

import sys
import types

import numpy as np

B = 4
S = 2048
D = 512
NHEAD = 8
DK = 64
NCORES = 8
MAX_REL = 255
NKT = S // 128   # 16 k-tiles
NU = S // 512    # 4 q-units
NG = NKT // 2    # 8 k-tile pairs


def _install_axon_hooks():
    """Provide antenv.axon_hooks (missing in this image) so bass_utils'
    trace path can be used; harmless when tracing is off."""
    try:
        import antenv
    except ImportError:
        return
    try:
        from antenv.axon_hooks import get_axon_ntff_profile_hook  # noqa: F401
        return
    except ImportError:
        pass
    hook = None
    try:
        from trn_agent_boot.trn_boot import _ntff_profile_via_ctypes
        hook = _ntff_profile_via_ctypes("/opt/axon/libaxon_pjrt.so")
    except Exception:
        hook = None
    m = types.ModuleType("antenv.axon_hooks")
    m.get_axon_ntff_profile_hook = lambda: hook
    m.set_axon_ntff_profile_hook = lambda h: None
    sys.modules["antenv.axon_hooks"] = m
    antenv.axon_hooks = m


_install_axon_hooks()

import concourse.bass as bass  # noqa: E402
import concourse.bacc as bacc  # noqa: E402
import concourse.mybir as mybir  # noqa: E402
from concourse import tile  # noqa: E402
from concourse.bass_utils import run_bass_kernel_spmd  # noqa: E402
from concourse.vector_clock import ScopedClock as _ScopedClock  # noqa: E402

f32 = mybir.dt.float32
f32r = mybir.dt.float32r
bf16 = mybir.dt.bfloat16
f16 = mybir.dt.float16
fp8 = mybir.dt.float8e4
AF = mybir.ActivationFunctionType
DR = mybir.MatmulPerfMode.DoubleRow


def _patched_drain_and_barrier(self, tick_clock, wait_clock):
    # walrus in this container rejects >2 sem waits on one instruction; emit
    # the tail-drain waits as standalone wait instructions instead.
    nc = self.nc
    dummy = mybir.InstNoOp(name="drain-wait-probe", engine=mybir.EngineType.SP)
    wait_clock.add_sem_waits(dummy, _ScopedClock({None: tick_clock.global_clock}))
    handles = {h.name: h for h in self.sems.allocated().values()}
    si = dummy.sync_info
    for w in (si.on_wait if si is not None else []):
        nc.sync.wait_ge(handles[w.ant_name], w.wait_value)
    nc.sync.drain()
    nc.all_engine_barrier()
    popped = nc._tile_sem_poison_stack.pop()
    assert popped is self._sem_poison
    nc.clear_and_free_semaphores(list(self.sems.allocated().values()))
    nc.all_engine_barrier()


tile.TileContext._drain_and_barrier = _patched_drain_and_barrier


def _delta(t, u):
    return 128 * t - 512 * u


def _cls(t, u):
    d = _delta(t, u)
    if d <= -384:
        return 1  # whole block clamps to table[0]
    if d >= 768:
        return 2  # whole block clamps to table[510]
    return 0      # in-band: needs the Toeplitz block


def _didx(t, u):
    return (_delta(t, u) + 256) // 128  # 0..7 for in-band blocks


def build_program():
    nc = bacc.Bacc()

    # xk/xq arrive as 8 column-chunk tensors each so projections can start on
    # partial data; layout [d_model(128-part c-tile), chunk cols]
    xqT = nc.declare_dram_parameter("xqT", [D, S], f16, isOutput=False)
    xkT = nc.declare_dram_parameter("xkT", [D, S], f16, isOutput=False)
    xvT = nc.declare_dram_parameter("xvT", [D, S], f16, isOutput=False)
    wq = nc.declare_dram_parameter("wq", [128, 4, 256], f16, isOutput=False)
    wk = nc.declare_dram_parameter("wk", [128, 4, 256], f16, isOutput=False)
    wv = nc.declare_dram_parameter("wv", [128, 4, 256], f16, isOutput=False)
    wo = nc.declare_dram_parameter("wo", [128, 4, 512], f16, isOutput=False)
    ebd = nc.declare_dram_parameter("eb", [128, 4, 8, 512], f16, isOutput=False)
    cbd = nc.declare_dram_parameter("cb", [128, 4, 3], f32, isOutput=False)
    outd = nc.declare_dram_parameter("out", [S, D], f16, isOutput=True)
    import os as _os
    _dbg = _os.environ.get("KDEBUG", "0") == "1"
    if _dbg:
        dbg_qt = nc.declare_dram_parameter("dbg_qt", [128, 2, S], f16, isOutput=True)
        dbg_kt = nc.declare_dram_parameter("dbg_kt", [128, 2, S], f16, isOutput=True)
        dbg_v8 = nc.declare_dram_parameter("dbg_v8", [128, 4, NG, 2, 80], f16, isOutput=True)
        dbg_vf = nc.declare_dram_parameter("dbg_vf", [128, NKT, 4, 65], f16, isOutput=True)
        dbg_pt = nc.declare_dram_parameter("dbg_pt", [128, 4, 1024], f16, isOutput=True)
        dbg_ctx = nc.declare_dram_parameter("dbg_ctx", [65, 2, 512], f32, isOutput=True)

    with tile.TileContext(nc) as tc:
        with (
            tc.tile_pool(name="sb", bufs=1) as pool,
            tc.tile_pool(name="xt", bufs=1) as xpool,
            tc.tile_pool(name="pt", bufs=4) as ppool,
            tc.tile_pool(name="cxp", bufs=3) as cpool,
        ):
            # ---- persistent SBUF tiles -------------------------------------
            wq_sb = pool.tile([128, 4, 256], f16, tag="wq")
            wk_sb = pool.tile([128, 4, 256], f16, tag="wk")
            wv_sb = pool.tile([128, 4, 256], f16, tag="wv")
            wo_sb = pool.tile([128, 4, 512], f16, tag="wo")
            eb_sb = pool.tile([128, 4, 8, 512], f16, tag="eb")
            cb_sb = pool.tile([128, 4, 3], f32, tag="cb")
            qtz = [pool.tile([128, 2, S], f16, tag=f"qtz{a}", name=f"qtz{a}")
                   for a in range(2)]
            kt_sb = pool.tile([128, 2, S], f16, tag="kt")
            # f16 V for in-band groups: [s(128), k-tile, head, 65]
            vf_sb = pool.tile([128, NKT, 4, 65], f16, tag="vf")
            ones_c = pool.tile([128, 128], f16, tag="ones")
            linvb_pad = pool.tile([128, 512], f16, tag="lpad")
            cx_slots = [pool.tile([128, 512], f16, tag=f"cxs{i}",
                                  name=f"cxs{i}") for i in range(8)]
            warm = pool.tile([128, 16], f32, tag="warm")

            # ---- input DMAs over 2 rings (sync HWDGE, gpsimd SWDGE) ----
            # The scalar (ACT) ring is kept DMA-free: DMA_DIRECT2D costs
            # ~1.7us of ACT queue time each and would delay the exp stream.
            kts = [xpool.tile([128, S], f16, tag=f"xk{ct}", name=f"xk{ct}")
                   for ct in range(4)]
            qts = [xpool.tile([128, S], f16, tag=f"xq{ct}", name=f"xq{ct}")
                   for ct in range(4)]
            xts = [xpool.tile([128, S], f16, tag=f"xv{ct}", name=f"xv{ct}")
                   for ct in range(4)]
            nc.sync.dma_start(cb_sb[:], cbd[:])
            nc.sync.dma_start(wk_sb[:], wk[:])
            nc.sync.dma_start(wq_sb[:], wq[:])
            for ct in range(2):
                for half in range(2):
                    nc.sync.dma_start(
                        kts[ct][:, half * 1024:(half + 1) * 1024],
                        xkT[ct * 128:(ct + 1) * 128,
                            half * 1024:(half + 1) * 1024])
            for ct in range(4):
                nc.sync.dma_start(xts[ct][:, 768:1536],
                                  xvT[ct * 128:(ct + 1) * 128, 768:1536])
            nc.sync.dma_start(eb_sb[:, :, 2:4, :], ebd[:, :, 2:4, :])
            nc.sync.dma_start(eb_sb[:, :, 4:6, :], ebd[:, :, 4:6, :])
            for sc in range(1, 4):
                for ct in range(4):
                    nc.sync.dma_start(
                        qts[ct][:, sc * 512:(sc + 1) * 512],
                        xqT[ct * 128:(ct + 1) * 128, sc * 512:(sc + 1) * 512])
            nc.gpsimd.dma_start(wv_sb[:], wv[:])
            for ct in range(4):
                nc.gpsimd.dma_start(qts[ct][:, 0:512],
                                    xqT[ct * 128:(ct + 1) * 128, 0:512])
            for ct in range(2, 4):
                for half in range(2):
                    nc.gpsimd.dma_start(
                        kts[ct][:, half * 1024:(half + 1) * 1024],
                        xkT[ct * 128:(ct + 1) * 128,
                            half * 1024:(half + 1) * 1024])
            for lo, hi in ((1536, 2048), (0, 768)):
                for ct in range(4):
                    nc.gpsimd.dma_start(xts[ct][:, lo:hi],
                                        xvT[ct * 128:(ct + 1) * 128, lo:hi])
            nc.gpsimd.dma_start(eb_sb[:, :, 6:8, :], ebd[:, :, 6:8, :])
            nc.gpsimd.dma_start(wo_sb[:], wo[:])
            nc.gpsimd.dma_start(eb_sb[:, :, 0:2, :], ebd[:, :, 0:2, :])

            nc.vector.memset(qtz[0][:], 0.0)
            nc.vector.memset(qtz[1][:], 0.0)
            for i in range(8):
                nc.vector.memset(cx_slots[i][64:128, :], 0.0)
            nc.vector.memset(ones_c[:], 1.0)
            nc.vector.memset(linvb_pad[:], 0.0)
            nc.vector.memset(warm[:], 0.0)
            nc.scalar.activation(warm[:], warm[:], AF.Exp, bias=0.0, scale=1.0)
            # ones columns of both V layouts (written once, V copies fill 0:64)
            nc.vector.memset(vf_sb[:, :, :, 64:65], 1.0)

            # ---- phase A prefix: K hp0 and Q sc0/hp0 only; everything else
            # (K hp1, Q hp1/sc0, V projection) is deferred into the phase-B
            # pipeline as pending units so the PE warms up and stays warm.
            with tc.tile_pool(name="pa", bufs=6, space="PSUM") as pa:
                for hp in range(2):
                    pks = [pa.tile([128, 512], f32, tag="pa",
                                   name=f"pk{hp}_{sc}") for sc in range(4)]
                    for ct in range(4):
                        for sc in range(4):
                            nc.tensor.matmul(
                                pks[sc][:],
                                lhsT=wk_sb[:, ct, hp * 128:(hp + 1) * 128],
                                rhs=kts[ct][:, sc * 512:(sc + 1) * 512],
                                start=(ct == 0), stop=(ct == 3),
                            )
                    for sc in range(4):
                        nc.vector.tensor_copy(
                            kt_sb[:, hp, sc * 512:(sc + 1) * 512], pks[sc][:])
                pq = pa.tile([128, 512], f32, tag="pa", name="pq0_0")
                for ct in range(4):
                    nc.tensor.matmul(
                        pq[:],
                        lhsT=wq_sb[:, ct, 0:128],
                        rhs=qts[ct][:, 0:512],
                        start=(ct == 0), stop=(ct == 3),
                    )
                nc.vector.tensor_copy(qtz[0][0:64, 0, 0:512], pq[0:64, :])
                nc.vector.tensor_copy(qtz[1][64:128, 0, 0:512], pq[64:128, :])

            pv = {}

            def _v_unit(sts):
                def emit(pa2):
                    for st in sts:
                        if st % 2 == 0:
                            pv[st // 2] = pa2.tile(
                                [128, 512], f32, tag="pa2",
                                name=f"pv{st // 2}")
                        for ct in range(4):
                            nc.tensor.matmul(
                                pv[st // 2][:, (st % 2) * 256:
                                            (st % 2) * 256 + 256],
                                lhsT=xts[ct][:, st * 128:(st + 1) * 128],
                                rhs=wv_sb[:, ct, :],
                                start=(ct == 0), stop=(ct == 3),
                            )
                        psl = (pv[st // 2][:, (st % 2) * 256:
                                           (st % 2) * 256 + 256]
                               .rearrange("p (h x) -> p h x", x=64))
                        nc.vector.tensor_copy(vf_sb[:, st, :, 0:64], psl)
                return emit

            def _k_unit(sc):
                def emit(pa2):
                    pk = pa2.tile([128, 512], f32, tag="pa2",
                                  name=f"pk1_{sc}")
                    for ct in range(4):
                        nc.tensor.matmul(
                            pk[:],
                            lhsT=wk_sb[:, ct, 128:256],
                            rhs=kts[ct][:, sc * 512:(sc + 1) * 512],
                            start=(ct == 0), stop=(ct == 3),
                        )
                    nc.vector.tensor_copy(
                        kt_sb[:, 1, sc * 512:(sc + 1) * 512], pk[:])
                return emit

            # pending work units
            # pending work units (emitted between pipeline items so the PE
            # FIFO always has runnable work): remaining Q projections first.
            pend = []       # ready work (norm/oproj of previous blocks)
            pend_late = {}  # {(u, hp): [units]} Q-projections, DMA-gated

            def _q_proj_unit(hp, sc):
                def emit(pa2):
                    pq = pa2.tile([128, 512], f32, tag="pa2", name=f"pq{hp}_{sc}")
                    for ct in range(4):
                        nc.tensor.matmul(
                            pq[:],
                            lhsT=wq_sb[:, ct, hp * 128:(hp + 1) * 128],
                            rhs=qts[ct][:, sc * 512:(sc + 1) * 512],
                            start=(ct == 0), stop=(ct == 3),
                        )
                    nc.vector.tensor_copy(
                        qtz[0][0:64, hp, sc * 512:(sc + 1) * 512], pq[0:64, :])
                    nc.vector.tensor_copy(
                        qtz[1][64:128, hp, sc * 512:(sc + 1) * 512],
                        pq[64:128, :])
                return emit

            pend_late[(0, 1)] = [_q_proj_unit(0, 1), _q_proj_unit(1, 1)]
            pend_late[(1, 0)] = [_q_proj_unit(0, 2), _q_proj_unit(1, 2)]
            pend_late[(1, 1)] = [_q_proj_unit(0, 3), _q_proj_unit(1, 3)]
            pend.extend([
                _v_unit((6, 7)), _v_unit((8, 9)), _v_unit((10, 11)),
                _q_proj_unit(1, 0), _v_unit((12, 13)), _v_unit((14, 15)),
                _v_unit((0, 1)), _v_unit((2, 3)), _v_unit((4, 5)),
            ])

            with (
                tc.tile_pool(name="sc", bufs=2, space="PSUM") as scp,
                tc.tile_pool(name="c1", bufs=1, space="PSUM") as c1p,
                tc.tile_pool(name="pa2", bufs=2, space="PSUM") as pa2,
            ):
                for u in range(NU):
                    for hp in range(2):
                        if u == NU - 1 and hp == 1:
                            gorder = sorted(range(NG),
                                            key=lambda g: (_cls(2 * g, u) != 0,
                                                           g))
                        else:
                            gorder = sorted(range(NG),
                                            key=lambda g: (_cls(2 * g, u) == 0,
                                                           g))
                        _dbg_gs = ()
                        if _dbg and u == 0 and hp == 0:
                            _dbg_gs = (gorder[0],
                                       next(g for g in gorder
                                            if _cls(2 * g, u) == 0))
                        ctxp = [c1p.tile([65, 512], f32, tag=f"cp{i}", bufs=1,
                                         name=f"ctxp{u}{hp}{i}") for i in range(2)]
                        nav = [0, 0]
                        n_steps = {0: 2 * NG, 1: 2 * NG}

                        # pipeline items: (g, ah); AV lags scores by LOOK items
                        last_blk = (u == NU - 1 and hp == 1)
                        if last_blk:
                            items = ([(g, 0) for g in gorder]
                                     + [(g, 1) for g in gorder])
                        else:
                            items = [(g, ah) for g in gorder
                                     for ah in range(2)]
                        LOOK = 3
                        stage = {}

                        # ---- tail: normalization, deferred into next block --
                        def _norm_unit(u, hp, ah, ctxp_t):
                            def emit(pa2):
                                ctxf = cpool.tile([65, 512], f32, tag="ctxf",
                                                  bufs=3)
                                nc.vector.tensor_copy(ctxf[:], ctxp_t[:])
                                lp0 = cpool.tile([1, 512], f32, tag="lp0",
                                                 bufs=2)
                                nc.gpsimd.dma_start(lp0[:], ctxf[64:65, :])
                                lrow = cpool.tile([1, 512], f32, tag="lr",
                                                  bufs=2)
                                nc.vector.reciprocal_approx_fast(lrow[:], lp0[:])
                                nc.vector.tensor_copy(linvb_pad[0:1, :],
                                                      lrow[:])
                                bc = pa2.tile([128, 512], f32, tag="pa2")
                                nc.tensor.matmul(
                                    bc[:], lhsT=ones_c[:], rhs=linvb_pad[:],
                                    start=True, stop=True,
                                )
                                lh_ = 2 * hp + ah
                                cxn = cx_slots[lh_ + 4 * (u % 2)]
                                nc.vector.tensor_mul(cxn[0:64, :], bc[0:64, :],
                                                     ctxf[0:64, :])
                                cx_store[(u, lh_)] = cxn
                            return emit


                        def emit_scores(g, ah):
                            sct = scp.tile([128, 1024], f32, tag="sc",
                                           name=f"sct{u}{hp}{g}{ah}")
                            for ti in range(2):
                                t = 2 * g + ti
                                nc.tensor.matmul(
                                    sct[:, ti * 512:(ti + 1) * 512],
                                    lhsT=kt_sb[:, hp, t * 128:(t + 1) * 128],
                                    rhs=qtz[ah][:, hp,
                                                u * 512:(u + 1) * 512],
                                    start=True, stop=True,
                                )
                            cls = _cls(2 * g, u)
                            lh = 2 * hp + ah
                            if cls == 0:
                                ptf = ppool.tile([128, 1024], f16, tag="ptf",
                                                 bufs=5)
                                nc.scalar.activation(
                                    ptf[:], sct[:], AF.Exp,
                                    bias=cb_sb[:, lh, 0:1], scale=1.0,
                                )
                                src = ppool.tile([128, 1024], f16, tag="src",
                                                 bufs=5)
                                d0 = _didx(2 * g, u)
                                nc.vector.tensor_mul(
                                    src[:],
                                    ptf[:],
                                    eb_sb[:, lh, d0:d0 + 2, :].rearrange(
                                        "p a b -> p (a b)"),
                                )
                                stage[(g, ah)] = src
                            else:
                                src = ppool.tile([128, 1024], f16, tag="src",
                                                 bufs=5)
                                nc.scalar.activation(
                                    src[:], sct[:], AF.Exp,
                                    bias=cb_sb[:, lh, cls:cls + 1], scale=1.0,
                                )
                                stage[(g, ah)] = src
                            if _dbg and u == 0 and hp == 0 and g in _dbg_gs:
                                st_ = stage[(g, ah)]
                                slot = (0 if _cls(2 * g, u) else 2) + ah
                                dcp = ppool.tile([128, 1024], f16, tag="dcp",
                                                 bufs=2)
                                nc.vector.tensor_copy(dcp[:], st_[:])
                                nc.sync.dma_start(dbg_pt[:, slot, :], dcp[:])

                        def emit_av(g, ah):
                            src = stage.pop((g, ah))
                            lh = 2 * hp + ah
                            for ti in range(2):
                                t = 2 * g + ti
                                nav[ah] += 1
                                nc.tensor.matmul(
                                    ctxp[ah][:],
                                    lhsT=vf_sb[:, t, lh, :],
                                    rhs=src[:, ti * 512:(ti + 1) * 512],
                                    start=(nav[ah] == 1),
                                    stop=(nav[ah] == n_steps[ah]),
                                )
                            if last_blk and nav[ah] == n_steps[ah]:
                                _norm_unit(u, hp, ah, ctxp[ah])(pa2)

                        late = pend_late.pop((u, hp), [])
                        for i, (g, ah) in enumerate(items):
                            emit_scores(g, ah)
                            if pend:
                                pend.pop(0)(pa2)
                            elif late and i >= 10:
                                late.pop(0)(pa2)
                            if i >= LOOK:
                                emit_av(*items[i - LOOK])
                        while late:
                            late.pop(0)(pa2)
                        for i in range(len(items) - LOOK, len(items)):
                            emit_av(*items[i])

                        if _dbg and u == 0 and hp == 0:
                            for ah in range(2):
                                dcf = cpool.tile([65, 512], f32, tag="dcf",
                                                 bufs=2)
                                nc.vector.tensor_copy(dcf[:], ctxp[ah][:])
                                nc.sync.dma_start(dbg_ctx[:, ah, :], dcf[:])

                        if not last_blk:
                            for ah in range(2):
                                pend.append(_norm_unit(u, hp, ah, ctxp[ah]))

                        if hp == 1:
                            def _oproj_unit(u, qs):
                                def emit(pa2):
                                    po = pa2.tile([128, 512], f32, tag="pa2",
                                                  name=f"po{u}{qs}")
                                    for lh in range(4):
                                        nc.tensor.matmul(
                                            po[:],
                                            lhsT=cx_store[(u, lh)][
                                                :, qs * 128:(qs + 1) * 128],
                                            rhs=wo_sb[:, lh, :],
                                            start=(lh == 0), stop=(lh == 3),
                                        )
                                    ob = cpool.tile([128, 512], f16, tag="ob",
                                                    bufs=2)
                                    if u == NU - 1:
                                        nc.scalar.copy(ob[:], po[:])
                                    else:
                                        nc.vector.tensor_copy(ob[:], po[:])
                                    ring = nc.gpsimd if qs % 2 else nc.sync
                                    ring.dma_start(
                                        outd[u * 512 + qs * 128:
                                             u * 512 + (qs + 1) * 128, :],
                                        ob[:],
                                    )
                                return emit
                            for qs in range(4):
                                if last_blk:
                                    _oproj_unit(u, qs)(pa2)
                                else:
                                    pend_late.setdefault((u + 1, 0), []).append(
                                        _oproj_unit(u, qs))

                # flush any remaining tail units
                while pend:
                    pend.pop(0)(pa2)

                if _dbg:
                    nc.sync.dma_start(dbg_qt[:], qt_sb[:])
                    nc.sync.dma_start(dbg_kt[:], kt_sb[:])
                    nc.sync.dma_start(dbg_vf[:], vf_sb[:])
                    pass

    nc.compile()
    return nc


cx_store = {}

_PROGRAM = None


def _get_program():
    global _PROGRAM
    if _PROGRAM is None:
        _PROGRAM = build_program()
    return _PROGRAM


_IDX = None


def _idx_table():
    global _IDX
    if _IDX is None:
        p = np.arange(128)[:, None]
        f = np.arange(512)[None, :]
        blocks = []
        for didx in range(8):
            delta = didx * 128 - 256
            blocks.append(np.clip(delta + p - f + 255, 0, 510))
        _IDX = np.stack(blocks, axis=0)  # [8, 128, 512]
    return _IDX


def kernel(**inputs):
    query = np.asarray(inputs["query"], dtype=np.float32)
    key = np.asarray(inputs["key"], dtype=np.float32)
    value = np.asarray(inputs["value"], dtype=np.float32)
    mask = np.asarray(inputs["mask"])
    Wq = np.asarray(inputs["Wq"], dtype=np.float32)
    Wk = np.asarray(inputs["Wk"], dtype=np.float32)
    Wv = np.asarray(inputs["Wv"], dtype=np.float32)
    Wo = np.asarray(inputs["Wo"], dtype=np.float32)
    bo = np.asarray(inputs["bo"], dtype=np.float32)
    rel_bias = np.asarray(inputs["rel_bias"], dtype=np.float32)

    if not np.all(mask != 0):
        raise NotImplementedError("kernel assumes an all-ones attention mask")

    nc = _get_program()
    idx = _idx_table()
    scale = np.float32(1.0 / np.sqrt(DK))

    in_maps = []
    for c in range(NCORES):
        b = c // 2
        hbase = (c % 2) * 4
        rows = slice(hbase * 64, (hbase + 4) * 64)

        wq_arr = np.ascontiguousarray(
            (Wq[rows, :] * scale).T.reshape(4, 128, 256).swapaxes(0, 1))
        wk_arr = np.ascontiguousarray(
            Wk[rows, :].T.reshape(4, 128, 256).swapaxes(0, 1))
        wv_arr = np.ascontiguousarray(
            Wv[rows, :].T.reshape(4, 128, 256).swapaxes(0, 1))

        # exp-shift: softmax is invariant to exp(s - C); C keeps the fp8e4
        # exp outputs well below the 448 saturation point.
        C = np.float32(3.0)
        wo_arr = np.zeros((128, 4, 512), dtype=np.float32)
        eb_arr = np.empty((128, 4, 8, 512), dtype=np.float16)
        cb_arr = np.zeros((128, 4, 3), dtype=np.float32)
        for lh in range(4):
            g = hbase + lh
            wo_arr[0:64, lh, :] = Wo[:, g * 64:(g + 1) * 64].T
            tbl = rel_bias[g]
            eb_arr[:, lh, :, :] = np.exp(tbl)[idx].transpose(1, 0, 2)
            cb_arr[:, lh, 0] = -C
            cb_arr[:, lh, 1] = tbl[0] - C
            cb_arr[:, lh, 2] = tbl[510] - C

        bf = np.float16
        in_maps.append({
            "xqT": np.ascontiguousarray(query[b].T).astype(bf),
            "xkT": np.ascontiguousarray(key[b].T).astype(bf),
            "xvT": np.ascontiguousarray(value[b].T).astype(bf),
            "wq": wq_arr.astype(bf), "wk": wk_arr.astype(bf),
            "wv": wv_arr.astype(bf), "wo": wo_arr.astype(bf),
            "eb": eb_arr, "cb": cb_arr,
        })

    res = run_bass_kernel_spmd(nc, in_maps, list(range(NCORES)), trace=False)

    out = np.zeros((B, S, D), dtype=np.float32)
    for c in range(NCORES):
        out[c // 2] += res.results[c]["out"]
    out += bo[None, None, :]
    return out


# revision 4
# speedup vs baseline: 1.0227x; 1.0227x over previous
"""Trainium2 Bass kernel for nn_MultiHeadAttention_34162169872901.

MultiHeadAttention (B=4, S=2048, d_model=512, 8 heads, d_k=64) with a
relative-position bias table (511 entries, clamp +-255) and an all-ones mask.

Sharding (8 NeuronCores): core c handles batch b = c//2 and 4 of the 8 heads
(c%2 selects the head half); the host sums the two partial outputs per batch.

Design (~198us vs 264us baseline):
  - Pacing engine is the Scalar/ACT exp stream (128 activations of
    [128, 1024] over PSUM, ~1.2us each incl. semaphore overhead).  The
    scores->exp->(bias mul)->AV chain is software-pipelined per
    (k-tile-pair, head) item with the AV matmuls lagging LOOK=3 items so the
    PE and ACT engines overlap instead of alternating.
  - Every matmul runs in the full 128x128 array mode: per-head K=64 score
    contractions, the 1/l broadcast, and the d_k=64 O-projection are all
    zero-padded to K=128 (zeroed Q tiles / cx slots / all-ones stationary),
    because switching between row/col-tiled and full PE modes costs ~90ns
    per transition (~30us total when scores/AV/O-proj alternate).
  - Non-critical work (V projection, Q projections for later q-chunks, the
    previous block's normalization and O-projection) is emitted as pending
    units between pipeline items, keeping the PE FIFO full without blocking
    it on late DMAs (Q-proj/O-proj units flush late in the following block).
  - exp(s - 3) keeps f16 softmax sums in range (softmax is shift-invariant;
    the denominator comes from a 65th all-ones V column, its reciprocal is
    broadcast across partitions by an all-ones matmul).
  - Input DMAs ride 2 rings (sync HWDGE + gpsimd SWDGE) in first-use order
    with K chunks half-interleaved; the scalar/ACT ring stays DMA-free (each
    DMA instr costs ~1.7us of ACT queue time).  Outputs alternate rings; the
    last block runs in-band groups first with its tail inlined and
    ACT-assisted output casts.
  - All-f16 data path (f32 PSUM accumulation).  fp8e4 DoubleRow AV was tried
    and works but costs ~3% output error (over the 2e-2 gate), so it was
    removed.
"""

import sys
import types

import numpy as np

B = 4
S = 2048
D = 512
NHEAD = 8
DK = 64
NCORES = 8
MAX_REL = 255
NKT = S // 128   # 16 k-tiles
NU = S // 512    # 4 q-units
NG = NKT // 2    # 8 k-tile pairs


def _install_axon_hooks():
    """Provide antenv.axon_hooks (missing in this image) so bass_utils'
    trace path can be used; harmless when tracing is off."""
    try:
        import antenv
    except ImportError:
        return
    try:
        from antenv.axon_hooks import get_axon_ntff_profile_hook  # noqa: F401
        return
    except ImportError:
        pass
    hook = None
    try:
        from trn_agent_boot.trn_boot import _ntff_profile_via_ctypes
        hook = _ntff_profile_via_ctypes("/opt/axon/libaxon_pjrt.so")
    except Exception:
        hook = None
    m = types.ModuleType("antenv.axon_hooks")
    m.get_axon_ntff_profile_hook = lambda: hook
    m.set_axon_ntff_profile_hook = lambda h: None
    sys.modules["antenv.axon_hooks"] = m
    antenv.axon_hooks = m


_install_axon_hooks()

import concourse.bass as bass  # noqa: E402
import concourse.bacc as bacc  # noqa: E402
import concourse.mybir as mybir  # noqa: E402
from concourse import tile  # noqa: E402
from concourse.bass_utils import run_bass_kernel_spmd  # noqa: E402
from concourse.vector_clock import ScopedClock as _ScopedClock  # noqa: E402

f32 = mybir.dt.float32
f32r = mybir.dt.float32r
bf16 = mybir.dt.bfloat16
f16 = mybir.dt.float16
fp8 = mybir.dt.float8e4
AF = mybir.ActivationFunctionType
DR = mybir.MatmulPerfMode.DoubleRow


def _patched_drain_and_barrier(self, tick_clock, wait_clock):
    # walrus in this container rejects >2 sem waits on one instruction; emit
    # the tail-drain waits as standalone wait instructions instead.
    nc = self.nc
    dummy = mybir.InstNoOp(name="drain-wait-probe", engine=mybir.EngineType.SP)
    wait_clock.add_sem_waits(dummy, _ScopedClock({None: tick_clock.global_clock}))
    handles = {h.name: h for h in self.sems.allocated().values()}
    si = dummy.sync_info
    for w in (si.on_wait if si is not None else []):
        nc.sync.wait_ge(handles[w.ant_name], w.wait_value)
    nc.sync.drain()
    nc.all_engine_barrier()
    popped = nc._tile_sem_poison_stack.pop()
    assert popped is self._sem_poison
    nc.clear_and_free_semaphores(list(self.sems.allocated().values()))
    nc.all_engine_barrier()


tile.TileContext._drain_and_barrier = _patched_drain_and_barrier


def _delta(t, u):
    return 128 * t - 512 * u


def _cls(t, u):
    d = _delta(t, u)
    if d <= -384:
        return 1  # whole block clamps to table[0]
    if d >= 768:
        return 2  # whole block clamps to table[510]
    return 0      # in-band: needs the Toeplitz block


def _didx(t, u):
    return (_delta(t, u) + 256) // 128  # 0..7 for in-band blocks


def build_program():
    nc = bacc.Bacc()

    # xk/xq arrive as 8 column-chunk tensors each so projections can start on
    # partial data; layout [d_model(128-part c-tile), chunk cols]
    xqT = nc.declare_dram_parameter("xqT", [D, S], f16, isOutput=False)
    xkT = nc.declare_dram_parameter("xkT", [D, S], f16, isOutput=False)
    xvT = nc.declare_dram_parameter("xvT", [D, S], f16, isOutput=False)
    wq = nc.declare_dram_parameter("wq", [128, 4, 256], f16, isOutput=False)
    wk = nc.declare_dram_parameter("wk", [128, 4, 256], f16, isOutput=False)
    wv = nc.declare_dram_parameter("wv", [128, 4, 256], f16, isOutput=False)
    wo = nc.declare_dram_parameter("wo", [128, 4, 512], f16, isOutput=False)
    ebd = nc.declare_dram_parameter("eb", [128, 4, 8, 512], f16, isOutput=False)
    cbd = nc.declare_dram_parameter("cb", [128, 4, 3], f32, isOutput=False)
    outd = nc.declare_dram_parameter("out", [S, D], f16, isOutput=True)
    import os as _os
    _dbg = _os.environ.get("KDEBUG", "0") == "1"
    if _dbg:
        dbg_qt = nc.declare_dram_parameter("dbg_qt", [128, 2, S], f16, isOutput=True)
        dbg_kt = nc.declare_dram_parameter("dbg_kt", [128, 2, S], f16, isOutput=True)
        dbg_v8 = nc.declare_dram_parameter("dbg_v8", [128, 4, NG, 2, 80], f16, isOutput=True)
        dbg_vf = nc.declare_dram_parameter("dbg_vf", [128, NKT, 4, 65], f16, isOutput=True)
        dbg_pt = nc.declare_dram_parameter("dbg_pt", [128, 4, 1024], f16, isOutput=True)
        dbg_ctx = nc.declare_dram_parameter("dbg_ctx", [65, 2, 512], f32, isOutput=True)

    with tile.TileContext(nc) as tc:
        with (
            tc.tile_pool(name="sb", bufs=1) as pool,
            tc.tile_pool(name="xt", bufs=1) as xpool,
            tc.tile_pool(name="pt", bufs=4) as ppool,
            tc.tile_pool(name="cxp", bufs=3) as cpool,
        ):
            # ---- persistent SBUF tiles -------------------------------------
            wq_sb = pool.tile([128, 4, 256], f16, tag="wq")
            wk_sb = pool.tile([128, 4, 256], f16, tag="wk")
            wv_sb = pool.tile([128, 4, 256], f16, tag="wv")
            wo_sb = pool.tile([128, 4, 512], f16, tag="wo")
            eb_sb = pool.tile([128, 4, 8, 512], f16, tag="eb")
            cb_sb = pool.tile([128, 4, 3], f32, tag="cb")
            qtz = [pool.tile([128, 2, S], f16, tag=f"qtz{a}", name=f"qtz{a}")
                   for a in range(2)]
            kt_sb = pool.tile([128, 2, S], f16, tag="kt")
            # f16 V for in-band groups: [s(128), k-tile, head, 65]
            vf_sb = pool.tile([128, NKT, 4, 65], f16, tag="vf")
            ones_c = pool.tile([128, 128], f16, tag="ones")
            linvb_pad = pool.tile([128, 512], f16, tag="lpad")
            cx_slots = [pool.tile([128, 512], f16, tag=f"cxs{i}",
                                  name=f"cxs{i}") for i in range(8)]
            warm = pool.tile([128, 16], f32, tag="warm")

            # ---- input DMAs over 2 rings (sync HWDGE, gpsimd SWDGE) ----
            # The scalar (ACT) ring is kept DMA-free: DMA_DIRECT2D costs
            # ~1.7us of ACT queue time each and would delay the exp stream.
            kts = [xpool.tile([128, S], f16, tag=f"xk{ct}", name=f"xk{ct}")
                   for ct in range(4)]
            qts = [xpool.tile([128, S], f16, tag=f"xq{ct}", name=f"xq{ct}")
                   for ct in range(4)]
            xts = [xpool.tile([128, S], f16, tag=f"xv{ct}", name=f"xv{ct}")
                   for ct in range(4)]
            nc.sync.dma_start(cb_sb[:], cbd[:])
            nc.sync.dma_start(wk_sb[:], wk[:])
            nc.sync.dma_start(wq_sb[:], wq[:])
            for half in range(2):
                for ct in range(2):
                    nc.sync.dma_start(
                        kts[ct][:, half * 1024:(half + 1) * 1024],
                        xkT[ct * 128:(ct + 1) * 128,
                            half * 1024:(half + 1) * 1024])
            for ct in range(4):
                nc.sync.dma_start(xts[ct][:, 768:1536],
                                  xvT[ct * 128:(ct + 1) * 128, 768:1536])
            nc.sync.dma_start(eb_sb[:, :, 2:4, :], ebd[:, :, 2:4, :])
            nc.sync.dma_start(eb_sb[:, :, 4:6, :], ebd[:, :, 4:6, :])
            for sc in range(1, 4):
                for ct in range(4):
                    nc.sync.dma_start(
                        qts[ct][:, sc * 512:(sc + 1) * 512],
                        xqT[ct * 128:(ct + 1) * 128, sc * 512:(sc + 1) * 512])
            nc.gpsimd.dma_start(wv_sb[:], wv[:])
            for ct in range(4):
                nc.gpsimd.dma_start(qts[ct][:, 0:512],
                                    xqT[ct * 128:(ct + 1) * 128, 0:512])
            for half in range(2):
                for ct in range(2, 4):
                    nc.gpsimd.dma_start(
                        kts[ct][:, half * 1024:(half + 1) * 1024],
                        xkT[ct * 128:(ct + 1) * 128,
                            half * 1024:(half + 1) * 1024])
            for lo, hi in ((1536, 2048), (0, 768)):
                for ct in range(4):
                    nc.gpsimd.dma_start(xts[ct][:, lo:hi],
                                        xvT[ct * 128:(ct + 1) * 128, lo:hi])
            nc.gpsimd.dma_start(eb_sb[:, :, 6:8, :], ebd[:, :, 6:8, :])
            nc.gpsimd.dma_start(wo_sb[:], wo[:])
            nc.gpsimd.dma_start(eb_sb[:, :, 0:2, :], ebd[:, :, 0:2, :])

            nc.vector.memset(qtz[0][:], 0.0)
            nc.vector.memset(qtz[1][:], 0.0)
            for i in range(8):
                nc.vector.memset(cx_slots[i][64:128, :], 0.0)
            nc.vector.memset(ones_c[:], 1.0)
            nc.vector.memset(linvb_pad[:], 0.0)
            nc.vector.memset(warm[:], 0.0)
            nc.scalar.activation(warm[:], warm[:], AF.Exp, bias=0.0, scale=1.0)
            # ones columns of both V layouts (written once, V copies fill 0:64)
            nc.vector.memset(vf_sb[:, :, :, 64:65], 1.0)

            # ---- phase A prefix: K hp0 and Q sc0/hp0 only; everything else
            # (K hp1, Q hp1/sc0, V projection) is deferred into the phase-B
            # pipeline as pending units so the PE warms up and stays warm.
            with tc.tile_pool(name="pa", bufs=6, space="PSUM") as pa:
                for hp, scs in ((0, (0, 1)), (0, (2, 3)),
                                (1, (0, 1)), (1, (2, 3))):
                    pks = {sc: pa.tile([128, 512], f32, tag="pa",
                                       name=f"pk{hp}_{sc}") for sc in scs}
                    for ct in range(4):
                        for sc in scs:
                            nc.tensor.matmul(
                                pks[sc][:],
                                lhsT=wk_sb[:, ct, hp * 128:(hp + 1) * 128],
                                rhs=kts[ct][:, sc * 512:(sc + 1) * 512],
                                start=(ct == 0), stop=(ct == 3),
                            )
                    for sc in scs:
                        nc.vector.tensor_copy(
                            kt_sb[:, hp, sc * 512:(sc + 1) * 512], pks[sc][:])
                pq = pa.tile([128, 512], f32, tag="pa", name="pq0_0")
                for ct in range(4):
                    nc.tensor.matmul(
                        pq[:],
                        lhsT=wq_sb[:, ct, 0:128],
                        rhs=qts[ct][:, 0:512],
                        start=(ct == 0), stop=(ct == 3),
                    )
                nc.vector.tensor_copy(qtz[0][0:64, 0, 0:512], pq[0:64, :])
                nc.vector.tensor_copy(qtz[1][64:128, 0, 0:512], pq[64:128, :])

            pv = {}

            def _v_unit(sts):
                def emit(pa2):
                    for st in sts:
                        if st % 2 == 0:
                            pv[st // 2] = pa2.tile(
                                [128, 512], f32, tag="pa2",
                                name=f"pv{st // 2}")
                        for ct in range(4):
                            nc.tensor.matmul(
                                pv[st // 2][:, (st % 2) * 256:
                                            (st % 2) * 256 + 256],
                                lhsT=xts[ct][:, st * 128:(st + 1) * 128],
                                rhs=wv_sb[:, ct, :],
                                start=(ct == 0), stop=(ct == 3),
                            )
                        psl = (pv[st // 2][:, (st % 2) * 256:
                                           (st % 2) * 256 + 256]
                               .rearrange("p (h x) -> p h x", x=64))
                        nc.vector.tensor_copy(vf_sb[:, st, :, 0:64], psl)
                return emit

            def _k_unit(sc):
                def emit(pa2):
                    pk = pa2.tile([128, 512], f32, tag="pa2",
                                  name=f"pk1_{sc}")
                    for ct in range(4):
                        nc.tensor.matmul(
                            pk[:],
                            lhsT=wk_sb[:, ct, 128:256],
                            rhs=kts[ct][:, sc * 512:(sc + 1) * 512],
                            start=(ct == 0), stop=(ct == 3),
                        )
                    nc.vector.tensor_copy(
                        kt_sb[:, 1, sc * 512:(sc + 1) * 512], pk[:])
                return emit

            # pending work units
            # pending work units (emitted between pipeline items so the PE
            # FIFO always has runnable work): remaining Q projections first.
            pend = []       # ready work (norm/oproj of previous blocks)
            pend_late = {}  # {(u, hp): [units]} Q-projections, DMA-gated

            def _q_proj_unit(hp, sc):
                def emit(pa2):
                    pq = pa2.tile([128, 512], f32, tag="pa2", name=f"pq{hp}_{sc}")
                    for ct in range(4):
                        nc.tensor.matmul(
                            pq[:],
                            lhsT=wq_sb[:, ct, hp * 128:(hp + 1) * 128],
                            rhs=qts[ct][:, sc * 512:(sc + 1) * 512],
                            start=(ct == 0), stop=(ct == 3),
                        )
                    nc.vector.tensor_copy(
                        qtz[0][0:64, hp, sc * 512:(sc + 1) * 512], pq[0:64, :])
                    nc.vector.tensor_copy(
                        qtz[1][64:128, hp, sc * 512:(sc + 1) * 512],
                        pq[64:128, :])
                return emit

            pend_late[(0, 1)] = [_q_proj_unit(0, 1), _q_proj_unit(1, 1)]
            pend_late[(1, 0)] = [_q_proj_unit(0, 2), _q_proj_unit(1, 2)]
            pend_late[(1, 1)] = [_q_proj_unit(0, 3), _q_proj_unit(1, 3)]
            pend.extend([
                _v_unit((6, 7)), _v_unit((8, 9)), _v_unit((10, 11)),
                _q_proj_unit(1, 0), _v_unit((12, 13)), _v_unit((14, 15)),
                _v_unit((0, 1)), _v_unit((2, 3)), _v_unit((4, 5)),
            ])

            with (
                tc.tile_pool(name="sc", bufs=2, space="PSUM") as scp,
                tc.tile_pool(name="c1", bufs=1, space="PSUM") as c1p,
                tc.tile_pool(name="pa2", bufs=2, space="PSUM") as pa2,
            ):
                for u in range(NU):
                    for hp in range(2):
                        if u == NU - 1 and hp == 1:
                            gorder = sorted(range(NG),
                                            key=lambda g: (_cls(2 * g, u) != 0,
                                                           g))
                        else:
                            gorder = sorted(range(NG),
                                            key=lambda g: (_cls(2 * g, u) == 0,
                                                           g))
                        _dbg_gs = ()
                        if _dbg and u == 0 and hp == 0:
                            _dbg_gs = (gorder[0],
                                       next(g for g in gorder
                                            if _cls(2 * g, u) == 0))
                        ctxp = [c1p.tile([65, 512], f32, tag=f"cp{i}", bufs=1,
                                         name=f"ctxp{u}{hp}{i}") for i in range(2)]
                        nav = [0, 0]
                        n_steps = {0: 2 * NG, 1: 2 * NG}

                        # pipeline items: (g, ah); AV lags scores by LOOK items
                        last_blk = (u == NU - 1 and hp == 1)
                        if last_blk:
                            items = ([(g, 0) for g in gorder]
                                     + [(g, 1) for g in gorder])
                        else:
                            items = [(g, ah) for g in gorder
                                     for ah in range(2)]
                        LOOK = 3
                        stage = {}

                        # ---- tail: normalization, deferred into next block --
                        def _norm_unit(u, hp, ah, ctxp_t):
                            def emit(pa2):
                                ctxf = cpool.tile([65, 512], f32, tag="ctxf",
                                                  bufs=3)
                                nc.vector.tensor_copy(ctxf[:], ctxp_t[:])
                                lp0 = cpool.tile([1, 512], f32, tag="lp0",
                                                 bufs=2)
                                nc.gpsimd.dma_start(lp0[:], ctxf[64:65, :])
                                lrow = cpool.tile([1, 512], f32, tag="lr",
                                                  bufs=2)
                                nc.vector.reciprocal_approx_fast(lrow[:], lp0[:])
                                nc.vector.tensor_copy(linvb_pad[0:1, :],
                                                      lrow[:])
                                bc = pa2.tile([128, 512], f32, tag="pa2")
                                nc.tensor.matmul(
                                    bc[:], lhsT=ones_c[:], rhs=linvb_pad[:],
                                    start=True, stop=True,
                                )
                                lh_ = 2 * hp + ah
                                cxn = cx_slots[lh_ + 4 * (u % 2)]
                                nc.vector.tensor_mul(cxn[0:64, :], bc[0:64, :],
                                                     ctxf[0:64, :])
                                cx_store[(u, lh_)] = cxn
                            return emit


                        def emit_scores(g, ah):
                            sct = scp.tile([128, 1024], f32, tag="sc",
                                           name=f"sct{u}{hp}{g}{ah}")
                            for ti in range(2):
                                t = 2 * g + ti
                                nc.tensor.matmul(
                                    sct[:, ti * 512:(ti + 1) * 512],
                                    lhsT=kt_sb[:, hp, t * 128:(t + 1) * 128],
                                    rhs=qtz[ah][:, hp,
                                                u * 512:(u + 1) * 512],
                                    start=True, stop=True,
                                )
                            cls = _cls(2 * g, u)
                            lh = 2 * hp + ah
                            if cls == 0:
                                ptf = ppool.tile([128, 1024], f16, tag="ptf",
                                                 bufs=6)
                                nc.scalar.activation(
                                    ptf[:], sct[:], AF.Exp,
                                    bias=cb_sb[:, lh, 0:1], scale=1.0,
                                )
                                src = ppool.tile([128, 1024], f16, tag="src",
                                                 bufs=6)
                                d0 = _didx(2 * g, u)
                                nc.vector.tensor_mul(
                                    src[:],
                                    ptf[:],
                                    eb_sb[:, lh, d0:d0 + 2, :].rearrange(
                                        "p a b -> p (a b)"),
                                )
                                stage[(g, ah)] = src
                            else:
                                src = ppool.tile([128, 1024], f16, tag="src",
                                                 bufs=6)
                                nc.scalar.activation(
                                    src[:], sct[:], AF.Exp,
                                    bias=cb_sb[:, lh, cls:cls + 1], scale=1.0,
                                )
                                stage[(g, ah)] = src
                            if _dbg and u == 0 and hp == 0 and g in _dbg_gs:
                                st_ = stage[(g, ah)]
                                slot = (0 if _cls(2 * g, u) else 2) + ah
                                dcp = ppool.tile([128, 1024], f16, tag="dcp",
                                                 bufs=2)
                                nc.vector.tensor_copy(dcp[:], st_[:])
                                nc.sync.dma_start(dbg_pt[:, slot, :], dcp[:])

                        def emit_av(g, ah):
                            src = stage.pop((g, ah))
                            lh = 2 * hp + ah
                            for ti in range(2):
                                t = 2 * g + ti
                                nav[ah] += 1
                                nc.tensor.matmul(
                                    ctxp[ah][:],
                                    lhsT=vf_sb[:, t, lh, :],
                                    rhs=src[:, ti * 512:(ti + 1) * 512],
                                    start=(nav[ah] == 1),
                                    stop=(nav[ah] == n_steps[ah]),
                                )
                            if last_blk and nav[ah] == n_steps[ah]:
                                _norm_unit(u, hp, ah, ctxp[ah])(pa2)

                        late = pend_late.pop((u, hp), [])
                        first_blk = (u == 0 and hp == 0)
                        for i, (g, ah) in enumerate(items):
                            emit_scores(g, ah)
                            if pend and (first_blk or i % 2 == 1):
                                pend.pop(0)(pa2)
                            elif late and i >= 10:
                                late.pop(0)(pa2)
                            if i >= LOOK:
                                emit_av(*items[i - LOOK])
                        while late:
                            late.pop(0)(pa2)
                        for i in range(len(items) - LOOK, len(items)):
                            emit_av(*items[i])

                        if _dbg and u == 0 and hp == 0:
                            for ah in range(2):
                                dcf = cpool.tile([65, 512], f32, tag="dcf",
                                                 bufs=2)
                                nc.vector.tensor_copy(dcf[:], ctxp[ah][:])
                                nc.sync.dma_start(dbg_ctx[:, ah, :], dcf[:])

                        if not last_blk:
                            for ah in range(2):
                                pend.append(_norm_unit(u, hp, ah, ctxp[ah]))

                        if hp == 1:
                            def _oproj_unit(u, qs):
                                def emit(pa2):
                                    po = pa2.tile([128, 512], f32, tag="pa2",
                                                  name=f"po{u}{qs}")
                                    for lh in range(4):
                                        nc.tensor.matmul(
                                            po[:],
                                            lhsT=cx_store[(u, lh)][
                                                :, qs * 128:(qs + 1) * 128],
                                            rhs=wo_sb[:, lh, :],
                                            start=(lh == 0), stop=(lh == 3),
                                        )
                                    ob = cpool.tile([128, 512], f16, tag="ob",
                                                    bufs=2)
                                    if u == NU - 1:
                                        nc.scalar.copy(ob[:], po[:])
                                    else:
                                        nc.vector.tensor_copy(ob[:], po[:])
                                    ring = nc.gpsimd if qs % 2 else nc.sync
                                    ring.dma_start(
                                        outd[u * 512 + qs * 128:
                                             u * 512 + (qs + 1) * 128, :],
                                        ob[:],
                                    )
                                return emit
                            for qs in range(4):
                                if last_blk:
                                    _oproj_unit(u, qs)(pa2)
                                else:
                                    pend_late.setdefault((u + 1, 0), []).append(
                                        _oproj_unit(u, qs))

                # flush any remaining tail units
                while pend:
                    pend.pop(0)(pa2)

                if _dbg:
                    nc.sync.dma_start(dbg_qt[:], qt_sb[:])
                    nc.sync.dma_start(dbg_kt[:], kt_sb[:])
                    nc.sync.dma_start(dbg_vf[:], vf_sb[:])
                    pass

    nc.compile()
    return nc


cx_store = {}

_PROGRAM = None


def _get_program():
    global _PROGRAM
    if _PROGRAM is None:
        _PROGRAM = build_program()
    return _PROGRAM


_IDX = None


def _idx_table():
    global _IDX
    if _IDX is None:
        p = np.arange(128)[:, None]
        f = np.arange(512)[None, :]
        blocks = []
        for didx in range(8):
            delta = didx * 128 - 256
            blocks.append(np.clip(delta + p - f + 255, 0, 510))
        _IDX = np.stack(blocks, axis=0)  # [8, 128, 512]
    return _IDX


def kernel(**inputs):
    query = np.asarray(inputs["query"], dtype=np.float32)
    key = np.asarray(inputs["key"], dtype=np.float32)
    value = np.asarray(inputs["value"], dtype=np.float32)
    mask = np.asarray(inputs["mask"])
    Wq = np.asarray(inputs["Wq"], dtype=np.float32)
    Wk = np.asarray(inputs["Wk"], dtype=np.float32)
    Wv = np.asarray(inputs["Wv"], dtype=np.float32)
    Wo = np.asarray(inputs["Wo"], dtype=np.float32)
    bo = np.asarray(inputs["bo"], dtype=np.float32)
    rel_bias = np.asarray(inputs["rel_bias"], dtype=np.float32)

    if not np.all(mask != 0):
        raise NotImplementedError("kernel assumes an all-ones attention mask")

    nc = _get_program()
    idx = _idx_table()
    scale = np.float32(1.0 / np.sqrt(DK))

    in_maps = []
    for c in range(NCORES):
        b = c // 2
        hbase = (c % 2) * 4
        rows = slice(hbase * 64, (hbase + 4) * 64)

        wq_arr = np.ascontiguousarray(
            (Wq[rows, :] * scale).T.reshape(4, 128, 256).swapaxes(0, 1))
        wk_arr = np.ascontiguousarray(
            Wk[rows, :].T.reshape(4, 128, 256).swapaxes(0, 1))
        wv_arr = np.ascontiguousarray(
            Wv[rows, :].T.reshape(4, 128, 256).swapaxes(0, 1))

        # exp-shift: softmax is invariant to exp(s - C); C keeps the fp8e4
        # exp outputs well below the 448 saturation point.
        C = np.float32(3.0)
        wo_arr = np.zeros((128, 4, 512), dtype=np.float32)
        eb_arr = np.empty((128, 4, 8, 512), dtype=np.float16)
        cb_arr = np.zeros((128, 4, 3), dtype=np.float32)
        for lh in range(4):
            g = hbase + lh
            wo_arr[0:64, lh, :] = Wo[:, g * 64:(g + 1) * 64].T
            tbl = rel_bias[g]
            eb_arr[:, lh, :, :] = np.exp(tbl)[idx].transpose(1, 0, 2)
            cb_arr[:, lh, 0] = -C
            cb_arr[:, lh, 1] = tbl[0] - C
            cb_arr[:, lh, 2] = tbl[510] - C

        bf = np.float16
        in_maps.append({
            "xqT": np.ascontiguousarray(query[b].T).astype(bf),
            "xkT": np.ascontiguousarray(key[b].T).astype(bf),
            "xvT": np.ascontiguousarray(value[b].T).astype(bf),
            "wq": wq_arr.astype(bf), "wk": wk_arr.astype(bf),
            "wv": wv_arr.astype(bf), "wo": wo_arr.astype(bf),
            "eb": eb_arr, "cb": cb_arr,
        })

    res = run_bass_kernel_spmd(nc, in_maps, list(range(NCORES)), trace=False)

    out = np.zeros((B, S, D), dtype=np.float32)
    for c in range(NCORES):
        out[c // 2] += res.results[c]["out"]
    out += bo[None, None, :]
    return out
